# revision 1
# baseline (speedup 1.0000x reference)
"""Trainium2 Bass kernel for nn_GroupedQueryAttention_86380382257377.

Math note: the reference einsums collapse dramatically.
  scores = einsum('bqghd,bkgd->bqhg', q, k)  reduces over BOTH key pos and d,
  so only ksum[b,g,:] = sum_s k[b,s,g,:] is needed:
      scores[b,q,h,g] = x[b,q,:] . (Wq_blk[g,h] @ ksum[b,g]) / sqrt(D)
  out = einsum('bqhg,bsgd->bsgd', w, v) = wsum[b,g] * v[b,s,g,:]
  with wsum[b,g] = sum_{q,h} softmax_g(scores)[b,q,h,g], so
      out[b] = x[b] @ M[b] + cvec[b],
      M[b]   = sum_g wsum[b,g] * (Wv_g @ Wo_g),
      cvec[b]= sum_g wsum[b,g] * (bv_g @ Wo_g) + bo.

Sharding over 8 cores: core c owns group c for the Wq/Wk shards (one small
AllGather of the [D,B,H] wq_eff vectors) and owns output columns
[c*64,(c+1)*64) for the Wv@Wo / x@M stage (xT and WvT replicated).
"""

import numpy as np

B, S, D, G, H = 2, 2048, 512, 8, 4
N_CORES = 8
FSL = D // N_CORES  # 64 output columns per core
P = 128
DC = D // P  # 4
JC = S // P  # 16  (128-row score chunks over the full sequence)
SC = S // 512  # 4  (512-col moving chunks for the out matmul)
INV_SQRT_D = 1.0 / float(np.sqrt(D))

_cache = {}


def _build_nc():
    import concourse.bass as bass
    import concourse.mybir as mybir
    import concourse.tile as tile
    from concourse import bacc

    dt = mybir.dt.float32
    nc = bacc.Bacc(None, num_devices=N_CORES)

    # ---- kernel I/O (per-core views, host-prepared) ----
    xT_d = nc.dram_tensor("xT", [D, B, S], dt, kind="ExternalInput")      # [a, b, s]
    wvT_d = nc.dram_tensor("wvT", [G, D, D], dt, kind="ExternalInput")    # [g, e, a]
    wo_d = nc.dram_tensor("wo_sl", [G, D, FSL], dt, kind="ExternalInput")  # [g, e, f]
    wq_d = nc.dram_tensor("wqT", [D, H, D], dt, kind="ExternalInput")     # [e, h, a]
    wk_d = nc.dram_tensor("wk", [D, D], dt, kind="ExternalInput")         # [d, e]
    bk_d = nc.dram_tensor("bk_c", [D], dt, kind="ExternalInput")
    bq_d = nc.dram_tensor("bq_c", [H * D], dt, kind="ExternalInput")      # [h*512+e]
    bv_d = nc.dram_tensor("bv", [G * D], dt, kind="ExternalInput")
    bo_d = nc.dram_tensor("bo_sl", [FSL], dt, kind="ExternalInput")
    out_d = nc.dram_tensor("outT", [B, FSL, S], dt, kind="ExternalOutput")

    with tile.TileContext(nc) as tc:
        with (
            tc.tile_pool(name="sing", bufs=1) as sing,
            tc.tile_pool(name="wvp", bufs=2) as wvp,
            tc.tile_pool(name="pp", bufs=3, space="PSUM") as pp,
            tc.tile_pool(name="ppP", bufs=4, space="PSUM") as ppP,
            tc.tile_pool(name="dram", bufs=1, space="DRAM") as dram,
        ):
            # ---- persistent SBUF tiles ----
            x_sb = sing.tile([P, DC, B, S], dt)          # 8MB  [a_p, ac, b, s]
            wq_sb = sing.tile([P, DC, H, D], dt)         # 4MB  [e_p, ec, h, a]
            wo_sb = sing.tile([P, G, DC, FSL], dt)       # 1MB  [e_p, g, ec, f]
            wk_sb = sing.tile([P, DC, D], dt)            # 1MB  [d_p, dc, e]
            p_sb = sing.tile([P, DC, G, FSL], dt)        # 1MB  [a_p, ac, g, f]
            m_sb = sing.tile([P, DC, B, FSL], dt)        # .5MB [a_p, ac, b, f]
            out_sb = sing.tile([FSL, B, S], dt)          # 1MB  [f, b, s]
            wqe_all = sing.tile([P, DC, B, G, H], dt)    # .5MB [a_p, ac, b, g, h]
            s1_sb = sing.tile([P, B, JC, G, H], dt)      # .5MB scratch
            s2_sb = sing.tile([P, B, JC, G, H], dt)      # .5MB weights
            tmax = sing.tile([P, B, JC, H], dt)
            tden = sing.tile([P, B, JC, H], dt)
            trec = sing.tile([P, B, JC, H], dt)
            xs_sb = sing.tile([P, DC, B], dt)
            ksum_sb = sing.tile([P, DC, B], dt)          # [e_p, ec, b]
            bk_sb = sing.tile([P, DC], dt)
            bq_sb = sing.tile([P, DC, H], dt)            # [e_p, ec, h]
            bv_sb = sing.tile([P, G * DC], dt)           # [ge_p, ec32]
            bvs_sb = sing.tile([P, B, G * DC], dt)
            bo_sb = sing.tile([FSL, 1], dt)
            ones_sb = sing.tile([P, 1], dt)
            wsum_sb = sing.tile([1, B, G], dt)
            wsum_bc = sing.tile([P, B, G], dt)
            bqd_bc = sing.tile([P, B, G, H], dt)
            cvec_sb = sing.tile([FSL, B], dt)

            # ---- internal DRAM (collective bounce + broadcast) ----
            CHUNK = D * B * H + H * B  # 4096 wq_eff + 8 bq_dot
            wq_bounce = dram.tile([CHUNK], dt)
            wq_gath = dram.tile([G * CHUNK], dt)
            wsum_dd = dram.tile([B, G], dt)

            # ---- input DMAs (priority order = program order) ----
            for dc in range(DC):
                nc.sync.dma_start(
                    out=x_sb[:, dc, :, :], in_=xT_d[dc * P:(dc + 1) * P, :, :]
                )
            nc.sync.dma_start(
                out=wk_sb[:, :, :],
                in_=wk_d.rearrange("(dc p) e -> p dc e", p=P),
            )
            nc.sync.dma_start(
                out=bk_sb[:, :], in_=bk_d.rearrange("(ec p) -> p ec", p=P)
            )
            for h in range(H):
                nc.sync.dma_start(
                    out=bq_sb[:, :, h],
                    in_=bq_d[h * D:(h + 1) * D].rearrange("(ec p) -> p ec", p=P),
                )
            nc.sync.dma_start(
                out=wq_sb[:, :, :, :],
                in_=wq_d.rearrange("(ec p) h a -> p ec h a", p=P),
            )
            for g in range(G):
                nc.sync.dma_start(
                    out=wo_sb[:, g, :, :],
                    in_=wo_d[g, :, :].rearrange("(ec p) f -> p ec f", p=P),
                )
            nc.sync.dma_start(
                out=bv_sb[:, :], in_=bv_d.rearrange("(ec p) -> p ec", p=P)
            )
            nc.sync.dma_start(
                out=bo_sb[:, :], in_=bo_d.rearrange("(f o) -> f o", o=1)
            )
            nc.vector.memset(ones_sb[:, :], 1.0)

            # ---- A. xs[b,d] = sum_s x  (reduce innermost S) ----
            for dc in range(DC):
                nc.vector.tensor_reduce(
                    out=xs_sb[:, dc, :],
                    in_=x_sb[:, dc, :, :],
                    axis=mybir.AxisListType.X,
                    op=mybir.AluOpType.add,
                )

            # ---- B. ksumT[e,b] = Wk_c^T xs + S*bk  ----
            nc.vector.tensor_scalar_mul(bk_sb[:, :], bk_sb[:, :], float(S))
            psum_k = pp.tile([P, DC, B], dt, tag="big")
            for ec in range(DC):
                for dc in range(DC):
                    nc.tensor.matmul(
                        psum_k[:, ec, :],
                        lhsT=wk_sb[:, dc, ec * P:(ec + 1) * P],
                        rhs=xs_sb[:, dc, :],
                        start=(dc == 0),
                        stop=(dc == DC - 1),
                    )
            for ec in range(DC):
                nc.vector.tensor_scalar_add(
                    ksum_sb[:, ec, :], psum_k[:, ec, :], bk_sb[:, ec:ec + 1]
                )

            # ---- C. wq_eff[a,(b)] per (h, ac); bq_dot[h,b] ----
            psum_wq = pp.tile([P, H, DC, B], dt, tag="big")
            for h in range(H):
                for ac in range(DC):
                    for ec in range(DC):
                        nc.tensor.matmul(
                            psum_wq[:, h, ac, :],
                            lhsT=wq_sb[:, ec, h, ac * P:(ac + 1) * P],
                            rhs=ksum_sb[:, ec, :],
                            start=(ec == 0),
                            stop=(ec == DC - 1),
                        )
            psum_bqd = pp.tile([B, H], dt, tag="big")
            for ec in range(DC):
                nc.tensor.matmul(
                    psum_bqd[:, :],
                    lhsT=ksum_sb[:, ec, :],
                    rhs=bq_sb[:, ec, :],
                    start=(ec == 0),
                    stop=(ec == DC - 1),
                )
            # stage psum -> sbuf (layout [p, ac, b, h]) -> flat dram bounce
            wqe_loc = sing.tile([P, DC, B, H], dt)
            bqd_loc = sing.tile([B, H], dt)
            nc.vector.tensor_copy(
                wqe_loc[:, :, :, :].rearrange("p ac b h -> p h ac b"),
                psum_wq[:, :, :, :],
            )
            nc.vector.tensor_copy(bqd_loc[:, :], psum_bqd[:, :])
            nc.sync.dma_start(
                out=wq_bounce[0:D * B * H].rearrange(
                    "(p ac b h) -> p ac b h", p=P, ac=DC, b=B
                ),
                in_=wqe_loc[:, :, :, :],
            )
            nc.sync.dma_start(
                out=wq_bounce[D * B * H:CHUNK].rearrange("(b h) -> b h", b=B),
                in_=bqd_loc[:, :],
            )

            # ---- D. AllGather of (wq_eff, bq_dot) ----
            nc.gpsimd.collective_compute(
                "AllGather",
                mybir.AluOpType.bypass,
                replica_groups=[list(range(N_CORES))],
                ins=[wq_bounce[:].opt()],
                outs=[wq_gath[:].opt()],
            )

            # ---- E. spread gathered results ----
            gap = wq_gath[:]
            for b in range(B):
                for ac in range(DC):
                    nc.sync.dma_start(
                        out=wqe_all[:, ac, b, :, :].opt(),
                        in_=bass.AP(
                            tensor=gap.tensor,
                            offset=gap.offset + ac * B * H + b * H,
                            ap=[[DC * B * H, P], [CHUNK, G], [1, H]],
                        ),
                    )
            for b in range(B):
                nc.sync.dma_start(
                    out=bqd_bc[:, b, :, :],
                    in_=bass.AP(
                        tensor=gap.tensor,
                        offset=gap.offset + D * B * H + b * H,
                        ap=[[0, P], [CHUNK, G], [1, H]],
                    ),
                )
            nc.vector.tensor_scalar_mul(
                bqd_bc[:, :, :, :], bqd_bc[:, :, :, :], INV_SQRT_D
            )

            # ---- F. scores + softmax + wsum (full sequence, every core) ----
            for b in range(B):
                psum_s = pp.tile([P, JC, G, H], dt, tag="big")
                for j in range(JC):
                    for dc in range(DC):
                        nc.tensor.matmul(
                            psum_s[:, j, :, :],
                            lhsT=x_sb[:, dc, b, j * P:(j + 1) * P],
                            rhs=wqe_all[:, dc, b, :, :],
                            start=(dc == 0),
                            stop=(dc == DC - 1),
                        )
                # t = scores*inv_sqrt_d + bqd   (into s1)
                bqd_b = bqd_bc[:, b, :, :]
                nc.vector.scalar_tensor_tensor(
                    out=s1_sb[:, b, :, :, :],
                    in0=psum_s[:, :, :, :],
                    scalar=INV_SQRT_D,
                    in1=bass.AP(
                        tensor=bqd_b.tensor,
                        offset=bqd_b.offset,
                        ap=[list(bqd_b.ap[0]), [0, JC]] + list(bqd_b.ap[1:]),
                    ),
                    op0=mybir.AluOpType.mult,
                    op1=mybir.AluOpType.add,
                )
                # row max over g (innermost via stride permute)
                nc.vector.tensor_reduce(
                    out=tmax[:, b, :, :],
                    in_=s1_sb[:, b, :, :, :].rearrange("p j g h -> p j h g"),
                    axis=mybir.AxisListType.X,
                    op=mybir.AluOpType.max,
                )
                tmax_b = tmax[:, b, :, :]
                nc.vector.tensor_tensor(
                    out=s2_sb[:, b, :, :, :].rearrange("p j g h -> p j h g"),
                    in0=s1_sb[:, b, :, :, :].rearrange("p j g h -> p j h g"),
                    in1=bass.AP(
                        tensor=tmax_b.tensor,
                        offset=tmax_b.offset,
                        ap=list(tmax_b.ap) + [[0, G]],
                    ),
                    op=mybir.AluOpType.subtract,
                )
                nc.scalar.activation(
                    out=s1_sb[:, b, :, :, :],
                    in_=s2_sb[:, b, :, :, :],
                    func=mybir.ActivationFunctionType.Exp,
                )
                nc.vector.tensor_reduce(
                    out=tden[:, b, :, :],
                    in_=s1_sb[:, b, :, :, :].rearrange("p j g h -> p j h g"),
                    axis=mybir.AxisListType.X,
                    op=mybir.AluOpType.add,
                )
                nc.vector.reciprocal(trec[:, b, :, :], tden[:, b, :, :])
                trec_b = trec[:, b, :, :]
                nc.vector.tensor_tensor(
                    out=s2_sb[:, b, :, :, :].rearrange("p j g h -> p j h g"),
                    in0=s1_sb[:, b, :, :, :].rearrange("p j g h -> p j h g"),
                    in1=bass.AP(
                        tensor=trec_b.tensor,
                        offset=trec_b.offset,
                        ap=list(trec_b.ap) + [[0, G]],
                    ),
                    op=mybir.AluOpType.mult,
                )
                # wsum partial: ones^T @ weights -> [1, JC*G*H], reduce (j,h)
                psum_ws = pp.tile([1, JC * G * H], dt, tag="big")
                nc.tensor.matmul(
                    psum_ws[:, :],
                    lhsT=ones_sb[:, :],
                    rhs=s2_sb[:, b, :, :, :],
                    start=True,
                    stop=True,
                )
                # view [1, (g), (j), (h)] with g kept, (j,h) reduced
                psv = psum_ws[:, :].rearrange(
                    "p (j g h) -> p g j h", j=JC, g=G, h=H
                )
                nc.vector.tensor_reduce(
                    out=wsum_sb[:, b, :],
                    in_=psv,
                    axis=mybir.AxisListType.XY,
                    op=mybir.AluOpType.add,
                )

            # broadcast wsum to all partitions via DRAM
            nc.sync.dma_start(out=wsum_dd[:, :], in_=wsum_sb[:, :, :])
            wsrc = wsum_dd[:, :]
            nc.sync.dma_start(
                out=wsum_bc[:, :, :],
                in_=bass.AP(
                    tensor=wsrc.tensor,
                    offset=wsrc.offset,
                    ap=[[0, P]] + list(wsrc.ap),
                ),
            )

            # ---- G. P_g = Wv_g @ Wo_g[:, fsl]  (all groups, f-slice) ----
            for g in range(G):
                wv_g = wvp.tile([P, DC, D], dt)
                nc.sync.dma_start(
                    out=wv_g[:, :, :],
                    in_=wvT_d[g, :, :].rearrange("(ec p) a -> p ec a", p=P),
                )
                for ac in range(DC):
                    psum_p = ppP.tile([P, FSL], dt)
                    for ec in range(DC):
                        nc.tensor.matmul(
                            psum_p[:, :],
                            lhsT=wv_g[:, ec, ac * P:(ac + 1) * P],
                            rhs=wo_sb[:, g, ec, :],
                            start=(ec == 0),
                            stop=(ec == DC - 1),
                        )
                    nc.vector.tensor_copy(p_sb[:, ac, g, :], psum_p[:, :])

            # ---- H. M[b] = sum_g wsum[b,g] * P_g ----
            for b in range(B):
                nc.vector.tensor_scalar_mul(
                    m_sb[:, :, b, :], p_sb[:, :, 0, :], wsum_bc[:, b, 0:1]
                )
                for g in range(1, G):
                    nc.vector.scalar_tensor_tensor(
                        out=m_sb[:, :, b, :],
                        in0=p_sb[:, :, g, :],
                        scalar=wsum_bc[:, b, g:g + 1],
                        in1=m_sb[:, :, b, :],
                        op0=mybir.AluOpType.mult,
                        op1=mybir.AluOpType.add,
                    )

            # ---- I. cvec[b] = sum_g wsum[b,g] * (bv_g @ Wo_g[:,fsl]) + bo ----
            for b in range(B):
                wsb = wsum_bc[:, b, :]
                nc.vector.tensor_tensor(
                    out=bvs_sb[:, b, :].rearrange("p (g r) -> p g r", g=G),
                    in0=bv_sb[:, :].rearrange("p (g r) -> p g r", g=G),
                    in1=bass.AP(
                        tensor=wsb.tensor,
                        offset=wsb.offset,
                        ap=list(wsb.ap) + [[0, DC]],
                    ),
                    op=mybir.AluOpType.mult,
                )
                psum_cv = pp.tile([FSL, 1], dt, tag="big")
                for ec32 in range(G * DC):
                    nc.tensor.matmul(
                        psum_cv[:, :],
                        lhsT=wo_sb[:, ec32 // DC, ec32 % DC, :],
                        rhs=bvs_sb[:, b, ec32:ec32 + 1],
                        start=(ec32 == 0),
                        stop=(ec32 == G * DC - 1),
                    )
                nc.vector.tensor_tensor(
                    out=cvec_sb[:, b:b + 1],
                    in0=psum_cv[:, :],
                    in1=bo_sb[:, :],
                    op=mybir.AluOpType.add,
                )

            # ---- J. outT[b] = (x[b] @ M[b])^T + cvec ----
            for b in range(B):
                for sc in range(SC):
                    psum_o = pp.tile([FSL, 512], dt, tag="big")
                    for ac in range(DC):
                        nc.tensor.matmul(
                            psum_o[:, :],
                            lhsT=m_sb[:, ac, b, :],
                            rhs=x_sb[:, ac, b, sc * 512:(sc + 1) * 512],
                            start=(ac == 0),
                            stop=(ac == DC - 1),
                        )
                    nc.vector.tensor_scalar_add(
                        out_sb[:, b, sc * 512:(sc + 1) * 512],
                        psum_o[:, :],
                        cvec_sb[:, b:b + 1],
                    )
                nc.sync.dma_start(out=out_d[b, :, :], in_=out_sb[:, b, :])

    nc.compile()
    return nc


def kernel(x, Wq, bq, Wk, bk, Wv, bv, Wo, bo):
    from concourse.bass_utils import run_bass_kernel_spmd

    if "nc" not in _cache:
        _cache["nc"] = _build_nc()
    nc = _cache["nc"]

    x = np.ascontiguousarray(x, dtype=np.float32)
    xT = np.ascontiguousarray(x.transpose(2, 0, 1))                    # [D,B,S]
    wvT = np.ascontiguousarray(
        Wv.astype(np.float32).reshape(D, G, D).transpose(1, 2, 0)      # [g,e,a]
    )
    wo_r = Wo.astype(np.float32).reshape(G, D, D)
    wq_r = Wq.astype(np.float32).reshape(D, G, H, D)
    bq_r = np.ascontiguousarray(bq, dtype=np.float32)
    in_maps = []
    for c in range(N_CORES):
        fs = slice(c * FSL, (c + 1) * FSL)
        in_maps.append({
            "xT": xT,
            "wvT": wvT,
            "wo_sl": np.ascontiguousarray(wo_r[:, :, fs]),
            "wqT": np.ascontiguousarray(wq_r[:, c].transpose(2, 1, 0)),  # [e,h,a]
            "wk": np.ascontiguousarray(Wk[:, c * D:(c + 1) * D].astype(np.float32)),
            "bk_c": np.ascontiguousarray(bk[c * D:(c + 1) * D].astype(np.float32)),
            "bq_c": np.ascontiguousarray(bq_r[c * H * D:(c + 1) * H * D]),
            "bv": np.ascontiguousarray(bv, dtype=np.float32),
            "bo_sl": np.ascontiguousarray(bo[fs].astype(np.float32)),
        })
    res = run_bass_kernel_spmd(nc, in_maps, core_ids=list(range(N_CORES)))
    _cache["last_results"] = res
    outs = [r["outT"] for r in res.results]          # each [B, FSL, S]
    full = np.concatenate(outs, axis=1)              # [B, D, S]
    return np.ascontiguousarray(full.transpose(0, 2, 1)).astype(np.float32)



# revision 20
# speedup vs baseline: 2.1787x; 2.1787x over previous
"""Trainium2 Bass kernel for nn_GroupedQueryAttention_86380382257377.

Math: the reference einsums collapse —
  scores[b,q,h,g] = x[b,q,:] . wq_eff[b][:, g, h] + bqdot[b,g,h]
      with wq_eff[b][e,(g,h)] = sum_k Wq[e,(g,h),k] * ksum[b,g,k],
           ksum[b,g] = Wk_g^T xs[b] + S*bk_g,  xs[b] = sum_s x[b,s,:]
  weights = softmax_g(scores);  wsum[b,g] = sum_{q,h} weights
  out[b]  = x[b] @ M[b] + cvec[b],
      M[b] = sum_g wsum[b,g] * (Wv_g @ Wo_g),
      cvec[b] = sum_g wsum[b,g] * (bv_g @ Wo_g) + bo.

Sharding (8 cores): core c owns group c for the Wq/Wk shard (one small
fp16 AllGather of (wq_eff, bqdot)); x is replicated (fp16); the P = Wv@Wo
and x@M stages are column-sharded (64 output cols per core).  All heavy
matmuls and DMAs run in fp16 (PSUM accumulation stays fp32).
"""

import numpy as np

B, S, D, G, H = 2, 2048, 512, 8, 4
N_CORES = 8
FSL = D // N_CORES  # 64 output columns per core
P = 128
DC = D // P   # 4 chunks of the contraction dims
JC = S // P   # 16 score row-chunks
SC = S // 512  # 4 out column-chunks
NXCH = 4      # x DMA chunks (for overlapped xs reduction)
XCW = S // NXCH  # 512
INV_SQRT_D = 1.0 / float(np.sqrt(D))
CHUNK = D * B * H + B * H  # 4096 wq_eff + 8 bqdot  (fp16 elements)

_cache = {}


def _build_nc():
    import concourse.bass as bass
    import concourse.mybir as mybir
    import concourse.tile as tile
    from concourse import bacc

    f32 = mybir.dt.float32
    f16 = mybir.dt.float16
    nc = bacc.Bacc(None, num_devices=N_CORES)

    # ---- kernel I/O (host-prepared, fp16 unless noted) ----
    xT_d = nc.dram_tensor("xT16", [D, B, S], f16, kind="ExternalInput")     # [d,b,s]
    wk_d = nc.dram_tensor("wk16", [D, D], f16, kind="ExternalInput")        # [d,k]
    wq_d = nc.dram_tensor("wq16", [D, H, D], f16, kind="ExternalInput")     # [a,h,e]
    wvT_d = nc.dram_tensor("wvT16", [G, D, D], f16, kind="ExternalInput")   # [g,e,d]
    wo_d = nc.dram_tensor("wo16", [D, G, FSL], f16, kind="ExternalInput")   # [e,g,f]
    bq_d = nc.dram_tensor("bq16", [D, H], f16, kind="ExternalInput")        # [k,h]
    bk_d = nc.dram_tensor("bk32", [D], f32, kind="ExternalInput")
    bv_d = nc.dram_tensor("bv32", [D, G], f32, kind="ExternalInput")        # [e,g]
    bo_d = nc.dram_tensor("bo32", [FSL], f32, kind="ExternalInput")
    out_d = nc.dram_tensor("out16", [B, FSL, S], f16, kind="ExternalOutput")

    with tile.TileContext(nc) as tc:
        with (
            tc.tile_pool(name="sing", bufs=1) as sing,
            tc.tile_pool(name="pps", bufs=1, space="PSUM") as pps,
            tc.tile_pool(name="pp", bufs=2, space="PSUM") as pp,
            tc.tile_pool(name="pss", bufs=2, space="PSUM") as pss,
            tc.tile_pool(name="pws", bufs=1, space="PSUM") as pws,
            tc.tile_pool(name="ppo", bufs=2, space="PSUM") as ppo,
            tc.tile_pool(name="dram", bufs=1, space="DRAM") as dram,
        ):
            # ---- persistent SBUF tiles ----
            x_sb = sing.tile([P, DC, B, S], f16)          # 32KB/part
            red = sing.tile([P, DC, B, 1024], f16)        # xs tree scratch
            wv_sb = sing.tile([P, G, DC, D], f16)         # lhsT [e, d] per (g,ec)
            wq_sb = sing.tile([P, DC, H, D], f16)         # lhsT [a, e] per (h,ac)
            wk_sb = sing.tile([P, DC, D], f16)            # lhsT [d, k] per (kc,dc)
            wo_sb = sing.tile([P, DC, G, FSL], f16)       # rhs [e, f] per (g,ec)
            bq_sb = sing.tile([P, DC, H], f16)            # rhs [k, h]
            bk_sb = sing.tile([P, DC], f32)
            bkS_sb = sing.tile([P, DC], f32)
            bv_sb = sing.tile([P, DC, G], f32)
            bvw_sb = sing.tile([P, DC, G, B], f16)
            bo_sb = sing.tile([FSL, 1], f32)
            cvec_sb = sing.tile([FSL, B], f32)
            ones_sb = sing.tile([P, P], f16)
            xs32 = sing.tile([P, DC, B], f32)
            xs16 = sing.tile([P, DC, B], f16)
            ksum16 = sing.tile([P, DC, B], f16)
            wqe_loc = sing.tile([P, DC, B, H], f16)
            bqd_loc = sing.tile([B, H], f16)
            wqe_all = sing.tile([P, G, DC, B, H], f16)
            bqd_all = sing.tile([1, B, G, H], f16)
            s1_sb = sing.tile([P, B, JC, G, H], f32)      # exp(scores)
            den_sb = sing.tile([P, B, JC, H], f32)
            rec_sb = sing.tile([P, B, JC, H], f32)
            w16_sb = sing.tile([P, B, JC, G, H], f16)     # softmax weights
            wsum_sb = sing.tile([1, B, G], f32)
            ws16_sb = sing.tile([1, B * G], f16)
            wsum_bc = sing.tile([P, B, G], f32)
            p16 = sing.tile([P, G, DC, FSL], f16)         # P_g[:, fslice]
            m16 = sing.tile([P, B, DC, FSL], f16)         # M[b][:, fslice]
            out_sb = sing.tile([FSL, B, S], f16)

            # ---- internal DRAM (collective bounce) ----
            wq_bounce = dram.tile([CHUNK], f16)
            wq_gath = dram.tile([G * CHUNK], f16)

            nc.vector.memset(ones_sb[:, :], 1.0)

            # ---- input DMAs, ordered for the critical path:
            #      x chunks (xs tree), wk, wq  ->  AllGather chain
            #      wv, wo, biases              ->  P / cvec path
            nc.sync.dma_start(
                out=wq_sb[:, :, :, :], in_=wq_d.rearrange("(ac p) h e -> p ac h e", p=P)
            )
            for dc in range(DC):
                for hh in range(2):
                    nc.sync.dma_start(
                        out=x_sb[:, dc, :, hh * 1024:(hh + 1) * 1024],
                        in_=xT_d[dc * P:(dc + 1) * P, :, hh * 1024:(hh + 1) * 1024],
                    )
            nc.sync.dma_start(
                out=wk_sb[:, :, :], in_=wk_d.rearrange("(dc p) k -> p dc k", p=P)
            )
            nc.sync.dma_start(
                out=bk_sb[:, :], in_=bk_d.rearrange("(dc p) -> p dc", p=P)
            )
            nc.sync.dma_start(
                out=bq_sb[:, :, :], in_=bq_d.rearrange("(kc p) h -> p kc h", p=P)
            )

            # ---- A. xs[b,d] = sum_s x : fp16 halving tree per (dc, s-half) ----
            for dc in range(DC):
                for hh in range(2):
                    hb = hh * 1024
                    rb = hh * 512
                    nc.vector.tensor_tensor(
                        out=red[:, dc, :, rb:rb + 512],
                        in0=x_sb[:, dc, :, hb:hb + 512],
                        in1=x_sb[:, dc, :, hb + 512:hb + 1024],
                        op=mybir.AluOpType.add,
                    )
                    w = 256
                    while w >= 8:
                        nc.vector.tensor_tensor(
                            out=red[:, dc, :, rb:rb + w],
                            in0=red[:, dc, :, rb:rb + w],
                            in1=red[:, dc, :, rb + w:rb + 2 * w],
                            op=mybir.AluOpType.add,
                        )
                        w //= 2
            nc.vector.tensor_reduce(
                out=xs32[:, :, :],
                in_=red[:, :, :, :].rearrange(
                    "p dc b (hh o) -> p dc b hh o", hh=2
                )[:, :, :, :, 0:8],
                axis=mybir.AxisListType.XY,
                op=mybir.AluOpType.add,
            )
            nc.vector.tensor_copy(xs16[:, :, :], xs32[:, :, :])

            # ---- B. ksumT[k,b] = Wk_c^T xs + S*bk ----
            nc.vector.tensor_scalar_mul(bkS_sb[:, :], bk_sb[:, :], float(S))
            psmall = pps.tile([P, 512], f32, tag="small")
            psum_k = psmall[:, 0:8].rearrange("p (kc b) -> p kc b", kc=DC)
            for kc in range(DC):
                for dc in range(DC):
                    nc.tensor.matmul(
                        psum_k[:, kc, :],
                        lhsT=wk_sb[:, dc, kc * P:(kc + 1) * P],
                        rhs=xs16[:, dc, :],
                        start=(dc == 0),
                        stop=(dc == DC - 1),
                    )
            bk_b = bkS_sb[:, :]
            nc.vector.tensor_tensor(
                out=ksum16[:, :, :],
                in0=psum_k[:, :, :],
                in1=bass.AP(
                    tensor=bk_b.tensor, offset=bk_b.offset,
                    ap=list(bk_b.ap) + [[0, B]],
                ),
                op=mybir.AluOpType.add,
            )

            # ---- C. wq_eff[e,(b)] per (h, ec); bqdot[b,h]; scale; bounce ----
            psum_wq = psmall[:, 8:40].rearrange(
                "p (ec b h) -> p ec b h", ec=DC, b=B
            )
            for h in range(H):
                for ec in range(DC):
                    for kc in range(DC):
                        nc.tensor.matmul(
                            psum_wq[:, ec, :, h],
                            lhsT=wq_sb[:, kc, h, ec * P:(ec + 1) * P],
                            rhs=ksum16[:, kc, :],
                            start=(kc == 0),
                            stop=(kc == DC - 1),
                        )
            psum_bqd = psmall[0:B, 40:44]
            for kc in range(DC):
                nc.tensor.matmul(
                    psum_bqd[:, :],
                    lhsT=ksum16[:, kc, :],
                    rhs=bq_sb[:, kc, :],
                    start=(kc == 0),
                    stop=(kc == DC - 1),
                )
            nc.vector.tensor_scalar_mul(wqe_loc[:, :, :, :], psum_wq[:, :, :, :], INV_SQRT_D)
            nc.vector.tensor_scalar_mul(bqd_loc[:, :], psum_bqd[:, :], INV_SQRT_D)
            nc.sync.dma_start(
                out=wq_bounce[0:D * B * H].rearrange(
                    "(p ac b h) -> p ac b h", p=P, ac=DC, b=B
                ),
                in_=wqe_loc[:, :, :, :],
            )
            nc.sync.dma_start(
                out=wq_bounce[D * B * H:CHUNK].rearrange("(b h) -> b h", b=B),
                in_=bqd_loc[:, :],
            )

            # ---- D2. release the held weight DMAs (kept off the DMA engines
            #      until the AllGather bounce has been issued) ----
            for gp in range(4):
                nc.vector.tensor_copy(
                    wv_sb[0:1, 2 * gp, 0, 0:4], wqe_loc[0:1, 0, 0, 0:4]
                )
            nc.vector.tensor_copy(wo_sb[0:1, 0, 0, 0:4], wqe_loc[0:1, 0, 0, 0:4])
            for gp in range(4):
                nc.sync.dma_start(
                    out=wv_sb[:, 2 * gp:2 * gp + 2, :, :],
                    in_=wvT_d[2 * gp:2 * gp + 2, :, :].rearrange(
                        "g (ec p) d -> p g ec d", p=P
                    ),
                )
            nc.sync.dma_start(
                out=wo_sb[:, :, :, :],
                in_=wo_d.rearrange("(ec p) g f -> p ec g f", p=P),
            )
            nc.sync.dma_start(
                out=bv_sb[:, :, :], in_=bv_d.rearrange("(ec p) g -> p ec g", p=P)
            )
            nc.sync.dma_start(
                out=bo_sb[:, :], in_=bo_d.rearrange("(f o) -> f o", o=1)
            )

            # ---- D. AllGather of (wq_eff, bqdot), fp16 ----
            nc.gpsimd.collective_compute(
                "AllGather",
                mybir.AluOpType.bypass,
                replica_groups=[list(range(N_CORES))],
                ins=[wq_bounce[:].opt()],
                outs=[wq_gath[:].opt()],
            )

            # ---- E. spread gathered results ----
            gap = wq_gath[:]
            nc.sync.dma_start(
                out=wqe_all[:, :, :, :, :],
                in_=bass.AP(
                    tensor=gap.tensor,
                    offset=gap.offset,
                    ap=[[DC * B * H, P], [CHUNK, G], [1, DC * B * H]],
                ),
            )
            nc.sync.dma_start(
                out=bqd_all[:, :, :, :],
                in_=bass.AP(
                    tensor=gap.tensor,
                    offset=gap.offset + D * B * H,
                    ap=[[0, 1], [H, B], [CHUNK, G], [1, H]],
                ),
            )

            # ---- F. P_g = Wv_g @ Wo_g[:, fsl]  (all groups, f-slice) ----
            for g in range(G):
                psum_p = pp.tile([P, DC, FSL], f32, tag="pp")
                for dc in range(DC):
                    for ec in range(DC):
                        nc.tensor.matmul(
                            psum_p[:, dc, :],
                            lhsT=wv_sb[:, g, ec, dc * P:(dc + 1) * P],
                            rhs=wo_sb[:, ec, g, :],
                            start=(ec == 0),
                            stop=(ec == DC - 1),
                        )
                nc.scalar.activation(
                    out=p16[:, g, :, :],
                    in_=psum_p[:, :, :],
                    func=mybir.ActivationFunctionType.Copy,
                )

            # ---- G. scores + exp + softmax + wsum (full sequence) ----
            for b in range(B):
                psum_s = pss.tile([P, JC, G * H], f32, tag="ps")
                bq_b = bqd_all[:, b, :, :]
                nc.tensor.matmul(
                    psum_s[:, :, :],
                    lhsT=ones_sb[0:1, :],
                    rhs=bass.AP(
                        tensor=bq_b.tensor, offset=bq_b.offset,
                        ap=[list(bq_b.ap[0]), [0, JC]] + list(bq_b.ap[1:]),
                    ),
                    start=True,
                    stop=False,
                )
                for j in range(JC):
                    for dc in range(DC):
                        nc.tensor.matmul(
                            psum_s[:, j, :],
                            lhsT=x_sb[:, dc, b, j * P:(j + 1) * P],
                            rhs=wqe_all[:, :, dc, b, :],
                            start=False,
                            stop=(j == JC - 1 and dc == DC - 1),
                            skip_group_check=True,
                        )
                nc.scalar.activation(
                    out=s1_sb[:, b, :, :, :].rearrange("p j g h -> p j (g h)"),
                    in_=psum_s[:, :, :],
                    func=mybir.ActivationFunctionType.Exp,
                )
                nc.vector.tensor_reduce(
                    out=den_sb[:, b, :, :],
                    in_=s1_sb[:, b, :, :, :].rearrange("p j g h -> p j h g"),
                    axis=mybir.AxisListType.X,
                    op=mybir.AluOpType.add,
                )
                nc.vector.reciprocal(rec_sb[:, b, :, :], den_sb[:, b, :, :])
                rb = rec_sb[:, b, :, :]
                nc.vector.tensor_tensor(
                    out=w16_sb[:, b, :, :, :].rearrange("p j g h -> p j h g"),
                    in0=s1_sb[:, b, :, :, :].rearrange("p j g h -> p j h g"),
                    in1=bass.AP(
                        tensor=rb.tensor,
                        offset=rb.offset,
                        ap=list(rb.ap) + [[0, G]],
                    ),
                    op=mybir.AluOpType.mult,
                )
                psum_ws = pws.tile([1, JC * G * H], f32, tag="ws")
                nc.tensor.matmul(
                    psum_ws[:, :],
                    lhsT=ones_sb[:, 0:1],
                    rhs=w16_sb[:, b, :, :, :],
                    start=True,
                    stop=True,
                )
                nc.vector.tensor_reduce(
                    out=wsum_sb[:, b, :],
                    in_=psum_ws[:, :].rearrange("p (j g h) -> p g j h", j=JC, g=G),
                    axis=mybir.AxisListType.XY,
                    op=mybir.AluOpType.add,
                )

            # ---- H. broadcast wsum to all partitions via PE (per b) ----
            psum_wb = psmall[:, 44:60].rearrange("p (b g) -> p b g", b=B)
            for b in range(B):
                nc.vector.tensor_copy(ws16_sb[:, b * G:(b + 1) * G], wsum_sb[:, b, :])
                nc.tensor.matmul(
                    psum_wb[:, b, :], lhsT=ones_sb[0:1, :],
                    rhs=ws16_sb[:, b * G:(b + 1) * G],
                    start=True, stop=True,
                )
                nc.vector.tensor_copy(wsum_bc[:, b, :], psum_wb[:, b, :])

            # ---- H2. PE warm-up fillers: keep the tensor engine busy through
            #      the softmax/combine window so the out matmuls run at full
            #      clock (cheap redundant column-sums into a recycled bank) ----
            for _ in range(32):
                psum_fill = pss.tile([P, JC, G * H], f32, tag="ps")
                nc.tensor.matmul(
                    psum_fill[:, :, :],
                    lhsT=ones_sb[:, :],
                    rhs=w16_sb[:, 0, :, :, :],
                    start=True,
                    stop=True,
                )

            # ---- I. M[b] = sum_g wsum[b,g] * P_g ----
            for b in range(B):
                nc.vector.tensor_scalar(
                    out=m16[:, b, :, :],
                    in0=p16[:, 0, :, :],
                    scalar1=wsum_bc[:, b, 0:1],
                    scalar2=None,
                    op0=mybir.AluOpType.mult,
                )
                for g in range(1, G):
                    nc.vector.scalar_tensor_tensor(
                        out=m16[:, b, :, :],
                        in0=p16[:, g, :, :],
                        scalar=wsum_bc[:, b, g:g + 1],
                        in1=m16[:, b, :, :],
                        op0=mybir.AluOpType.mult,
                        op1=mybir.AluOpType.add,
                    )

            # ---- J. cvec[b] = sum_g wsum[b,g]*(bv_g @ Wo_g[:,fsl]) + bo ----
            for b in range(B):
                wb = wsum_bc[:, b, :]
                nc.vector.tensor_tensor(
                    out=bvw_sb[:, :, :, b],
                    in0=bv_sb[:, :, :],
                    in1=bass.AP(
                        tensor=wb.tensor,
                        offset=wb.offset,
                        ap=[list(wb.ap[0]), [0, DC]] + list(wb.ap[1:]),
                    ),
                    op=mybir.AluOpType.mult,
                )
            psum_cv = psmall[0:FSL, 60:62]
            for b in range(B):
                for ge in range(G * DC):
                    g, ec = ge // DC, ge % DC
                    nc.tensor.matmul(
                        psum_cv[:, b:b + 1],
                        lhsT=wo_sb[:, ec, g, :],
                        rhs=bvw_sb[:, ec, g, b:b + 1],
                        start=(ge == 0),
                        stop=(ge == G * DC - 1),
                    )
            bo_b = bo_sb[:, :]
            nc.vector.tensor_tensor(
                out=cvec_sb[:, :],
                in0=psum_cv[:, :],
                in1=bass.AP(
                    tensor=bo_b.tensor, offset=bo_b.offset,
                    ap=[list(bo_b.ap[0]), [0, B]],
                ),
                op=mybir.AluOpType.add,
            )

            # ---- K. outT[b] = (M[b]^T x[b]^T) + cvec ----
            for b in range(B):
                for sc in range(SC):
                    psum_o = ppo.tile([FSL, 512], f32, tag="po")
                    for dc in range(DC):
                        nc.tensor.matmul(
                            psum_o[:, :],
                            lhsT=m16[:, b, dc, :],
                            rhs=x_sb[:, dc, b, sc * 512:(sc + 1) * 512],
                            start=(dc == 0),
                            stop=(dc == DC - 1),
                        )
                    if sc % 2 == 0:
                        nc.scalar.activation(
                            out=out_sb[:, b, sc * 512:(sc + 1) * 512],
                            in_=psum_o[:, :],
                            func=mybir.ActivationFunctionType.Identity,
                            bias=cvec_sb[:, b:b + 1],
                        )
                    else:
                        nc.vector.tensor_scalar_add(
                            out_sb[:, b, sc * 512:(sc + 1) * 512],
                            psum_o[:, :],
                            cvec_sb[:, b:b + 1],
                        )
                    nc.sync.dma_start(
                        out=out_d[b, :, sc * 512:(sc + 1) * 512],
                        in_=out_sb[:, b, sc * 512:(sc + 1) * 512],
                    )

    nc.compile()
    return nc


def kernel(x, Wq, bq, Wk, bk, Wv, bv, Wo, bo):
    from concourse.bass_utils import run_bass_kernel_spmd

    if "nc" not in _cache:
        _cache["nc"] = _build_nc()
    nc = _cache["nc"]

    f16 = np.float16
    xT16 = np.ascontiguousarray(
        np.asarray(x, np.float32).transpose(2, 0, 1)).astype(f16)  # [d,b,s]
    wq_r = np.asarray(Wq, np.float32).reshape(D, G, H, D)
    wvT16 = np.ascontiguousarray(
        np.asarray(Wv, np.float32).reshape(D, G, D).transpose(1, 2, 0)
    ).astype(f16)                                                   # [g,e,d]
    wo_r = np.asarray(Wo, np.float32).reshape(G, D, D)
    bq_r = np.asarray(bq, np.float32).reshape(G, H, D)
    in_maps = []
    for c in range(N_CORES):
        fs = slice(c * FSL, (c + 1) * FSL)
        in_maps.append({
            "xT16": xT16,
            "wk16": np.ascontiguousarray(
                np.asarray(Wk, np.float32)[:, c * D:(c + 1) * D]).astype(f16),
            "wq16": np.ascontiguousarray(
                wq_r[:, c].transpose(2, 1, 0)).astype(f16),          # [a,h,e]
            "wvT16": wvT16,
            "wo16": np.ascontiguousarray(
                wo_r[:, :, fs].transpose(1, 0, 2)).astype(f16),      # [e,g,f]
            "bq16": np.ascontiguousarray(bq_r[c].T).astype(f16),     # [k,h]
            "bk32": np.ascontiguousarray(
                np.asarray(bk, np.float32)[c * D:(c + 1) * D]),
            "bv32": np.ascontiguousarray(
                np.asarray(bv, np.float32).reshape(G, D).T),         # [e,g]
            "bo32": np.ascontiguousarray(np.asarray(bo, np.float32)[fs]),
        })
    res = run_bass_kernel_spmd(nc, in_maps, core_ids=list(range(N_CORES)))
    _cache["last_results"] = res
    outs = [r["out16"] for r in res.results]         # each [B, FSL, S] f16
    full = np.concatenate(outs, axis=1)              # [B, D, S]
    return np.ascontiguousarray(full.transpose(0, 2, 1)).astype(np.float32)


# revision 29
# speedup vs baseline: 2.2541x; 1.0346x over previous
"""Trainium2 Bass kernel for nn_GroupedQueryAttention_86380382257377.

Math: the reference einsums collapse —
  scores[b,q,h,g] = x[b,q,:] . wq_eff[b][:, g, h] + bqdot[b,g,h]
      with wq_eff[b][e,(g,h)] = sum_k Wq[e,(g,h),k] * ksum[b,g,k],
           ksum[b,g] = Wk_g^T xs[b] + S*bk_g,  xs[b] = sum_s x[b,s,:]
  weights = softmax_g(scores);  wsum[b,g] = sum_{q,h} weights
  out[b]  = x[b] @ M[b] + cvec[b],
      M[b] = sum_g wsum[b,g] * (Wv_g @ Wo_g),
      cvec[b] = sum_g wsum[b,g] * (bv_g @ Wo_g) + bo.

Sharding (8 cores): core c owns group c for the Wq/Wk shard (one small
fp16 AllGather of (wq_eff, bqdot)); x is replicated (fp16); the P = Wv@Wo
and x@M stages are column-sharded (64 output cols per core).  All heavy
matmuls and DMAs run in fp16 (PSUM accumulation stays fp32).
"""

import numpy as np

B, S, D, G, H = 2, 2048, 512, 8, 4
N_CORES = 8
FSL = D // N_CORES  # 64 output columns per core
P = 128
DC = D // P   # 4 chunks of the contraction dims
JC = S // P   # 16 score row-chunks
SC = S // 512  # 4 out column-chunks
NXCH = 4      # x DMA chunks (for overlapped xs reduction)
XCW = S // NXCH  # 512
INV_SQRT_D = 1.0 / float(np.sqrt(D))
CHUNK = D * B * H + B * H  # 4096 wq_eff + 8 bqdot  (fp16 elements)

_cache = {}


def _build_nc():
    import concourse.bass as bass
    import concourse.mybir as mybir
    import concourse.tile as tile
    from concourse import bacc

    f32 = mybir.dt.float32
    f16 = mybir.dt.float16
    nc = bacc.Bacc(None, num_devices=N_CORES)

    # ---- kernel I/O (host-prepared, fp16 unless noted) ----
    xT_d = nc.dram_tensor("xT16", [D, B, S], f16, kind="ExternalInput")     # [d,b,s]
    wk_d = nc.dram_tensor("wk16", [D, D], f16, kind="ExternalInput")        # [d,k]
    wq_d = nc.dram_tensor("wq16", [D, H, D], f16, kind="ExternalInput")     # [a,h,e]
    wvT_d = nc.dram_tensor("wvT16", [G, D, D], f16, kind="ExternalInput")   # [g,e,d]
    wo_d = nc.dram_tensor("wo16", [D, G, FSL], f16, kind="ExternalInput")   # [e,g,f]
    bq_d = nc.dram_tensor("bq16", [D, H], f16, kind="ExternalInput")        # [k,h]
    bk_d = nc.dram_tensor("bk32", [D], f32, kind="ExternalInput")
    bv_d = nc.dram_tensor("bv32", [D, G], f32, kind="ExternalInput")        # [e,g]
    bo_d = nc.dram_tensor("bo32", [FSL], f32, kind="ExternalInput")
    out_d = nc.dram_tensor("out16", [B, JC, P, FSL], f16, kind="ExternalOutput")
    cv_d = nc.dram_tensor("cvec32", [FSL, B], f32, kind="ExternalOutput")

    with tile.TileContext(nc) as tc:
        with (
            tc.tile_pool(name="sing", bufs=1) as sing,
            tc.tile_pool(name="pps", bufs=1, space="PSUM") as pps,
            tc.tile_pool(name="pp", bufs=2, space="PSUM") as pp,
            tc.tile_pool(name="pss", bufs=2, space="PSUM") as pss,
            tc.tile_pool(name="pws", bufs=1, space="PSUM") as pws,
            tc.tile_pool(name="ppo", bufs=2, space="PSUM") as ppo,
            tc.tile_pool(name="dram", bufs=1, space="DRAM") as dram,
        ):
            # ---- persistent SBUF tiles ----
            x_sb = sing.tile([P, DC, B, S], f16)          # 32KB/part
            red = sing.tile([P, DC, B, 1024], f16)        # xs tree scratch
            wv_sb = sing.tile([P, G, DC, D], f16)         # lhsT [e, d] per (g,ec)
            wq_sb = sing.tile([P, DC, H, D], f16)         # lhsT [a, e] per (h,ac)
            wk_sb = sing.tile([P, DC, D], f16)            # lhsT [d, k] per (kc,dc)
            wo_sb = sing.tile([P, DC, G, FSL], f16)       # rhs [e, f] per (g,ec)
            bq_sb = sing.tile([P, DC, H], f16)            # rhs [k, h]
            bk_sb = sing.tile([P, DC], f32)
            bkS_sb = sing.tile([P, DC], f32)
            bv16 = sing.tile([P, DC, G], f16)
            bvo_sb = sing.tile([FSL, G], f32)
            bo_sb = sing.tile([FSL, 1], f32)
            cvec_sb = sing.tile([FSL, B], f32)
            ones_sb = sing.tile([P, P], f16)
            xs32 = sing.tile([P, DC, B], f32)
            xs16 = sing.tile([P, DC, B], f16)
            ksum16 = sing.tile([P, DC, B], f16)
            wqe_loc = sing.tile([P, DC, B, H], f16)
            bqd_loc = sing.tile([B, H], f16)
            wqe_all = sing.tile([P, G, DC, B, H], f16)
            bqd_all = sing.tile([1, B, G, H], f16)
            s1_sb = sing.tile([P, B, JC, G, H], f32)      # exp(scores)
            den_sb = sing.tile([P, B, JC, H], f32)
            rec_sb = sing.tile([P, B, JC, H], f32)
            w16_sb = sing.tile([P, B, JC, G, H], f16)     # softmax weights
            wsum_sb = sing.tile([1, B, G], f32)
            ws16_sb = sing.tile([1, B * G], f16)
            wsum_bc = sing.tile([P, B, G], f32)
            p16 = sing.tile([P, G, DC, FSL], f16)         # P_g[:, fslice]
            m16 = sing.tile([P, B, DC, FSL], f16)         # M[b][:, fslice]
            out_sb = sing.tile([P, JC, B, FSL], f16)

            # ---- internal DRAM (collective bounce) ----
            wq_bounce = dram.tile([CHUNK], f16)
            wq_gath = dram.tile([G * CHUNK], f16)

            nc.vector.memset(ones_sb[:, :], 1.0)

            # ---- input DMAs, ordered for the critical path:
            #      x chunks (xs tree), wk, wq  ->  AllGather chain
            #      wv, wo, biases              ->  P / cvec path
            nc.sync.dma_start(
                out=wq_sb[:, :, :, :], in_=wq_d.rearrange("(ac p) h e -> p ac h e", p=P)
            )
            for dc in range(DC):
                for hh in range(2):
                    nc.sync.dma_start(
                        out=x_sb[:, dc, :, hh * 1024:(hh + 1) * 1024],
                        in_=xT_d[dc * P:(dc + 1) * P, :, hh * 1024:(hh + 1) * 1024],
                    )
            nc.sync.dma_start(
                out=wk_sb[:, :, :], in_=wk_d.rearrange("(dc p) k -> p dc k", p=P)
            )
            nc.sync.dma_start(
                out=bk_sb[:, :], in_=bk_d.rearrange("(dc p) -> p dc", p=P)
            )
            nc.sync.dma_start(
                out=bq_sb[:, :, :], in_=bq_d.rearrange("(kc p) h -> p kc h", p=P)
            )

            # ---- A. xs[b,d] = sum_s x : fp16 halving tree per (dc, s-half) ----
            for dc in range(DC):
                for hh in range(2):
                    hb = hh * 1024
                    rb = hh * 512
                    nc.vector.tensor_tensor(
                        out=red[:, dc, :, rb:rb + 512],
                        in0=x_sb[:, dc, :, hb:hb + 512],
                        in1=x_sb[:, dc, :, hb + 512:hb + 1024],
                        op=mybir.AluOpType.add,
                    )
                    w = 256
                    while w >= 8:
                        nc.vector.tensor_tensor(
                            out=red[:, dc, :, rb:rb + w],
                            in0=red[:, dc, :, rb:rb + w],
                            in1=red[:, dc, :, rb + w:rb + 2 * w],
                            op=mybir.AluOpType.add,
                        )
                        w //= 2
            nc.vector.tensor_reduce(
                out=xs32[:, :, :],
                in_=red[:, :, :, :].rearrange(
                    "p dc b (hh o) -> p dc b hh o", hh=2
                )[:, :, :, :, 0:8],
                axis=mybir.AxisListType.XY,
                op=mybir.AluOpType.add,
            )
            nc.vector.tensor_copy(xs16[:, :, :], xs32[:, :, :])

            # ---- B. ksumT[k,b] = Wk_c^T xs + S*bk ----
            nc.vector.tensor_scalar_mul(bkS_sb[:, :], bk_sb[:, :], float(S))
            psmall = pps.tile([P, 512], f32, tag="small")
            psum_k = psmall[:, 0:8].rearrange("p (kc b) -> p kc b", kc=DC)
            for kc in range(DC):
                for dc in range(DC):
                    nc.tensor.matmul(
                        psum_k[:, kc, :],
                        lhsT=wk_sb[:, dc, kc * P:(kc + 1) * P],
                        rhs=xs16[:, dc, :],
                        start=(dc == 0),
                        stop=(dc == DC - 1),
                    )
            bk_b = bkS_sb[:, :]
            nc.vector.tensor_tensor(
                out=ksum16[:, :, :],
                in0=psum_k[:, :, :],
                in1=bass.AP(
                    tensor=bk_b.tensor, offset=bk_b.offset,
                    ap=list(bk_b.ap) + [[0, B]],
                ),
                op=mybir.AluOpType.add,
            )

            # ---- C. wq_eff[e,(b)] per (h, ec); bqdot[b,h]; scale; bounce ----
            psum_wq = psmall[:, 8:40].rearrange(
                "p (ec b h) -> p ec b h", ec=DC, b=B
            )
            for h in range(H):
                for ec in range(DC):
                    for kc in range(DC):
                        nc.tensor.matmul(
                            psum_wq[:, ec, :, h],
                            lhsT=wq_sb[:, kc, h, ec * P:(ec + 1) * P],
                            rhs=ksum16[:, kc, :],
                            start=(kc == 0),
                            stop=(kc == DC - 1),
                        )
            psum_bqd = psmall[0:B, 40:44]
            for kc in range(DC):
                nc.tensor.matmul(
                    psum_bqd[:, :],
                    lhsT=ksum16[:, kc, :],
                    rhs=bq_sb[:, kc, :],
                    start=(kc == 0),
                    stop=(kc == DC - 1),
                )
            nc.vector.tensor_scalar_mul(wqe_loc[:, :, :, :], psum_wq[:, :, :, :], INV_SQRT_D)
            nc.vector.tensor_scalar_mul(bqd_loc[:, :], psum_bqd[:, :], INV_SQRT_D)
            nc.sync.dma_start(
                out=wq_bounce[0:D * B * H].rearrange(
                    "(p ac b h) -> p ac b h", p=P, ac=DC, b=B
                ),
                in_=wqe_loc[:, :, :, :],
            )
            nc.sync.dma_start(
                out=wq_bounce[D * B * H:CHUNK].rearrange("(b h) -> b h", b=B),
                in_=bqd_loc[:, :],
            )

            # ---- D2. release the held weight DMAs (kept off the DMA engines
            #      until the AllGather bounce has been issued) ----
            for gp in range(4):
                nc.vector.tensor_copy(
                    wv_sb[0:1, 2 * gp, 0, 0:4], wqe_loc[0:1, 0, 0, 0:4]
                )
            nc.vector.tensor_copy(wo_sb[0:1, 0, 0, 0:4], wqe_loc[0:1, 0, 0, 0:4])
            for gp in range(4):
                nc.sync.dma_start(
                    out=wv_sb[:, 2 * gp:2 * gp + 2, :, :],
                    in_=wvT_d[2 * gp:2 * gp + 2, :, :].rearrange(
                        "g (ec p) d -> p g ec d", p=P
                    ),
                )
            nc.sync.dma_start(
                out=wo_sb[:, :, :, :],
                in_=wo_d.rearrange("(ec p) g f -> p ec g f", p=P),
            )
            nc.gpsimd.dma_start(
                out=bv16[:, :, :], in_=bv_d.rearrange("(ec p) g -> p ec g", p=P)
            )
            nc.sync.dma_start(
                out=bo_sb[:, :], in_=bo_d.rearrange("(f o) -> f o", o=1)
            )

            # ---- D. AllGather of (wq_eff, bqdot), fp16 ----
            nc.gpsimd.collective_compute(
                "AllGather",
                mybir.AluOpType.bypass,
                replica_groups=[list(range(N_CORES))],
                ins=[wq_bounce[:].opt()],
                outs=[wq_gath[:].opt()],
            )

            # ---- E. spread gathered results ----
            gap = wq_gath[:]
            nc.sync.dma_start(
                out=wqe_all[:, :, :, :, :],
                in_=bass.AP(
                    tensor=gap.tensor,
                    offset=gap.offset,
                    ap=[[DC * B * H, P], [CHUNK, G], [1, DC * B * H]],
                ),
            )
            nc.sync.dma_start(
                out=bqd_all[:, :, :, :],
                in_=bass.AP(
                    tensor=gap.tensor,
                    offset=gap.offset + D * B * H,
                    ap=[[0, 1], [H, B], [CHUNK, G], [1, H]],
                ),
            )

            # ---- F. P_g = Wv_g @ Wo_g[:, fsl]  (all groups, f-slice) ----
            for g in range(G):
                psum_p = pp.tile([P, DC, FSL], f32, tag="pp")
                for dc in range(DC):
                    for ec in range(DC):
                        nc.tensor.matmul(
                            psum_p[:, dc, :],
                            lhsT=wv_sb[:, g, ec, dc * P:(dc + 1) * P],
                            rhs=wo_sb[:, ec, g, :],
                            start=(ec == 0),
                            stop=(ec == DC - 1),
                        )
                nc.scalar.activation(
                    out=p16[:, g, :, :],
                    in_=psum_p[:, :, :],
                    func=mybir.ActivationFunctionType.Copy,
                )

            # ---- F2. bvo[f, g] = bv_g @ Wo_g[:, fsl]  (early) ----
            psum_bvo = psmall[0:FSL, 192:200]
            for g in range(G):
                for ec in range(DC):
                    nc.tensor.matmul(
                        psum_bvo[:, g:g + 1],
                        lhsT=wo_sb[:, ec, g, :],
                        rhs=bv16[:, ec, g:g + 1],
                        start=(ec == 0),
                        stop=(ec == DC - 1),
                    )
            nc.vector.tensor_copy(bvo_sb[:, :], psum_bvo[:, :])

            # ---- G. scores + exp + softmax + wsum (full sequence) ----
            for b in range(B):
                psum_s = pss.tile([P, JC, G * H], f32, tag="ps")
                bq_b = bqd_all[:, b, :, :]
                nc.tensor.matmul(
                    psum_s[:, :, :],
                    lhsT=ones_sb[0:1, :],
                    rhs=bass.AP(
                        tensor=bq_b.tensor, offset=bq_b.offset,
                        ap=[list(bq_b.ap[0]), [0, JC]] + list(bq_b.ap[1:]),
                    ),
                    start=True,
                    stop=False,
                )
                for j in range(JC):
                    for dc in range(DC):
                        nc.tensor.matmul(
                            psum_s[:, j, :],
                            lhsT=x_sb[:, dc, b, j * P:(j + 1) * P],
                            rhs=wqe_all[:, :, dc, b, :],
                            start=False,
                            stop=(j == JC - 1 and dc == DC - 1),
                            skip_group_check=True,
                        )
                nc.scalar.activation(
                    out=s1_sb[:, b, :, :, :].rearrange("p j g h -> p j (g h)"),
                    in_=psum_s[:, :, :],
                    func=mybir.ActivationFunctionType.Exp,
                )
                nc.vector.tensor_reduce(
                    out=den_sb[:, b, :, :],
                    in_=s1_sb[:, b, :, :, :].rearrange("p j g h -> p j h g"),
                    axis=mybir.AxisListType.X,
                    op=mybir.AluOpType.add,
                )
                nc.vector.reciprocal(rec_sb[:, b, :, :], den_sb[:, b, :, :])
                rb = rec_sb[:, b, :, :]
                nc.vector.tensor_tensor(
                    out=w16_sb[:, b, :, :, :].rearrange("p j g h -> p j h g"),
                    in0=s1_sb[:, b, :, :, :].rearrange("p j g h -> p j h g"),
                    in1=bass.AP(
                        tensor=rb.tensor,
                        offset=rb.offset,
                        ap=list(rb.ap) + [[0, G]],
                    ),
                    op=mybir.AluOpType.mult,
                )
                psum_ws = pws.tile([1, JC * G * H], f32, tag="ws")
                nc.tensor.matmul(
                    psum_ws[:, :],
                    lhsT=ones_sb[:, 0:1],
                    rhs=w16_sb[:, b, :, :, :],
                    start=True,
                    stop=True,
                )
                nc.vector.tensor_reduce(
                    out=wsum_sb[:, b, :],
                    in_=psum_ws[:, :].rearrange("p (j g h) -> p g j h", j=JC, g=G),
                    axis=mybir.AxisListType.XY,
                    op=mybir.AluOpType.add,
                )

            # ---- H. broadcast wsum to all partitions via PE (per b) ----
            psum_wb = psmall[:, 44:60].rearrange("p (b g) -> p b g", b=B)
            for b in range(B):
                nc.vector.tensor_copy(ws16_sb[:, b * G:(b + 1) * G], wsum_sb[:, b, :])
                nc.tensor.matmul(
                    psum_wb[:, b, :], lhsT=ones_sb[0:1, :],
                    rhs=ws16_sb[:, b * G:(b + 1) * G],
                    start=True, stop=True,
                )
                nc.vector.tensor_copy(wsum_bc[:, b, :], psum_wb[:, b, :])

            # ---- H2. PE warm-up fillers: keep the tensor engine busy through
            #      the softmax/combine window so the out matmuls run at full
            #      clock (cheap redundant column-sums into a recycled bank) ----
            for _ in range(32):
                psum_fill = pss.tile([P, JC, G * H], f32, tag="ps")
                nc.tensor.matmul(
                    psum_fill[:, :, :],
                    lhsT=ones_sb[:, :],
                    rhs=w16_sb[:, 0, :, :, :],
                    start=True,
                    stop=True,
                )

            # ---- I. M[b] = sum_g wsum[b,g] * P_g ----
            for b in range(B):
                nc.vector.tensor_scalar(
                    out=m16[:, b, :, :],
                    in0=p16[:, 0, :, :],
                    scalar1=wsum_bc[:, b, 0:1],
                    scalar2=None,
                    op0=mybir.AluOpType.mult,
                )
                for g in range(1, G):
                    nc.vector.scalar_tensor_tensor(
                        out=m16[:, b, :, :],
                        in0=p16[:, g, :, :],
                        scalar=wsum_bc[:, b, g:g + 1],
                        in1=m16[:, b, :, :],
                        op0=mybir.AluOpType.mult,
                        op1=mybir.AluOpType.add,
                    )

            # ---- J. cvec[b] = sum_g wsum[b,g]*bvo[:,g] + bo; flip to [1,(b f)] ----
            for b in range(B):
                nc.vector.scalar_tensor_tensor(
                    out=cvec_sb[:, b:b + 1],
                    in0=bvo_sb[:, 0:1],
                    scalar=wsum_bc[0:FSL, b, 0:1],
                    in1=bo_sb[:, :],
                    op0=mybir.AluOpType.mult,
                    op1=mybir.AluOpType.add,
                )
                for g in range(1, G):
                    nc.vector.scalar_tensor_tensor(
                        out=cvec_sb[:, b:b + 1],
                        in0=bvo_sb[:, g:g + 1],
                        scalar=wsum_bc[0:FSL, b, g:g + 1],
                        in1=cvec_sb[:, b:b + 1],
                        op0=mybir.AluOpType.mult,
                        op1=mybir.AluOpType.add,
                    )
            nc.sync.dma_start(out=cv_d[:, :], in_=cvec_sb[:, :])

            # ---- K. out[b, s, fsl] = x[b] @ M[b] + cvec  (s on partitions) ----
            for b in range(B):
                for hf in range(2):
                    psum_o = ppo.tile([P, 8, FSL], f32, tag="po")
                    for jj in range(8):
                        j = hf * 8 + jj
                        for dc in range(DC):
                            nc.tensor.matmul(
                                psum_o[:, jj, :],
                                lhsT=x_sb[:, dc, b, j * P:(j + 1) * P],
                                rhs=m16[:, b, dc, :],
                                start=(dc == 0),
                                stop=(dc == DC - 1),
                            )
                    if hf == 0:
                        nc.scalar.activation(
                            out=out_sb[:, hf * 8:(hf + 1) * 8, b, :],
                            in_=psum_o[:, :, :],
                            func=mybir.ActivationFunctionType.Identity,
                        )
                    else:
                        nc.vector.tensor_copy(
                            out_sb[:, hf * 8:(hf + 1) * 8, b, :],
                            psum_o[:, :, :],
                        )
                    nc.sync.dma_start(
                        out=out_d[b, hf * 8:(hf + 1) * 8, :, :].rearrange(
                            "j p f -> p j f"
                        ),
                        in_=out_sb[:, hf * 8:(hf + 1) * 8, b, :],
                    )

    nc.compile()
    return nc


def kernel(x, Wq, bq, Wk, bk, Wv, bv, Wo, bo):
    from concourse.bass_utils import run_bass_kernel_spmd

    if "nc" not in _cache:
        _cache["nc"] = _build_nc()
    nc = _cache["nc"]

    f16 = np.float16
    xT16 = np.ascontiguousarray(
        np.asarray(x, np.float32).transpose(2, 0, 1)).astype(f16)  # [d,b,s]
    wq_r = np.asarray(Wq, np.float32).reshape(D, G, H, D)
    wvT16 = np.ascontiguousarray(
        np.asarray(Wv, np.float32).reshape(D, G, D).transpose(1, 2, 0)
    ).astype(f16)                                                   # [g,e,d]
    wo_r = np.asarray(Wo, np.float32).reshape(G, D, D)
    bq_r = np.asarray(bq, np.float32).reshape(G, H, D)
    in_maps = []
    for c in range(N_CORES):
        fs = slice(c * FSL, (c + 1) * FSL)
        in_maps.append({
            "xT16": xT16,
            "wk16": np.ascontiguousarray(
                np.asarray(Wk, np.float32)[:, c * D:(c + 1) * D]).astype(f16),
            "wq16": np.ascontiguousarray(
                wq_r[:, c].transpose(2, 1, 0)).astype(f16),          # [a,h,e]
            "wvT16": wvT16,
            "wo16": np.ascontiguousarray(
                wo_r[:, :, fs].transpose(1, 0, 2)).astype(f16),      # [e,g,f]
            "bq16": np.ascontiguousarray(bq_r[c].T).astype(f16),     # [k,h]
            "bk32": np.ascontiguousarray(
                np.asarray(bk, np.float32)[c * D:(c + 1) * D]),
            "bv32": np.ascontiguousarray(
                np.asarray(bv, np.float32).reshape(G, D).T),         # [e,g]
            "bo32": np.ascontiguousarray(np.asarray(bo, np.float32)[fs]),
        })
    res = run_bass_kernel_spmd(nc, in_maps, core_ids=list(range(N_CORES)))
    _cache["last_results"] = res
    full = np.concatenate(
        [r["out16"].reshape(B, S, FSL) for r in res.results], axis=2
    ).astype(np.float32)                              # [B, S, D]
    cvec = np.concatenate(
        [r["cvec32"].T for r in res.results], axis=1
    )                                                 # [B, D]
    return full + cvec[:, None, :]


# revision 35
# speedup vs baseline: 2.2814x; 1.0121x over previous
"""Trainium2 Bass kernel for nn_GroupedQueryAttention_86380382257377.

Math: the reference einsums collapse —
  scores[b,q,h,g] = x[b,q,:] . wq_eff[b][:, g, h] + bqdot[b,g,h]
      with wq_eff[b][e,(g,h)] = sum_k Wq[e,(g,h),k] * ksum[b,g,k],
           ksum[b,g] = Wk_g^T xs[b] + S*bk_g,  xs[b] = sum_s x[b,s,:]
  weights = softmax_g(scores);  wsum[b,g] = sum_{q,h} weights
  out[b]  = x[b] @ M[b] + cvec[b],
      M[b] = sum_g wsum[b,g] * (Wv_g @ Wo_g),
      cvec[b] = sum_g wsum[b,g] * (bv_g @ Wo_g) + bo.

Sharding (8 cores): core c owns group c for the Wq/Wk shard (one small
fp16 AllGather of (wq_eff, bqdot)); x is replicated (fp16); the P = Wv@Wo
and x@M stages are column-sharded (64 output cols per core).  All heavy
matmuls and DMAs run in fp16 (PSUM accumulation stays fp32).
"""

import numpy as np

B, S, D, G, H = 2, 2048, 512, 8, 4
N_CORES = 8
FSL = D // N_CORES  # 64 output columns per core
P = 128
DC = D // P   # 4 chunks of the contraction dims
JC = S // P   # 16 score row-chunks
SC = S // 512  # 4 out column-chunks
NXCH = 4      # x DMA chunks (for overlapped xs reduction)
XCW = S // NXCH  # 512
INV_SQRT_D = 1.0 / float(np.sqrt(D))
CHUNK = D * B * H + B * H  # 4096 wq_eff + 8 bqdot  (fp16 elements)

_cache = {}


def _build_nc():
    import concourse.bass as bass
    import concourse.mybir as mybir
    import concourse.tile as tile
    from concourse import bacc

    f32 = mybir.dt.float32
    f16 = mybir.dt.float16
    nc = bacc.Bacc(None, num_devices=N_CORES)

    # ---- kernel I/O (host-prepared, fp16 unless noted) ----
    xT_d = nc.dram_tensor("xT16", [D, B, S], f16, kind="ExternalInput")     # [d,b,s]
    wk_d = nc.dram_tensor("wk16", [D, D], f16, kind="ExternalInput")        # [d,k]
    wq_d = nc.dram_tensor("wq16", [D, H, D], f16, kind="ExternalInput")     # [a,h,e]
    wvT_d = nc.dram_tensor("wvT16", [G, D, D], f16, kind="ExternalInput")   # [g,e,d]
    wo_d = nc.dram_tensor("wo16", [D, G, FSL], f16, kind="ExternalInput")   # [e,g,f]
    bq_d = nc.dram_tensor("bq16", [D, H], f16, kind="ExternalInput")        # [k,h]
    bk_d = nc.dram_tensor("bk32", [D], f32, kind="ExternalInput")
    bv_d = nc.dram_tensor("bv32", [D, G], f32, kind="ExternalInput")        # [e,g]
    bo_d = nc.dram_tensor("bo32", [FSL], f32, kind="ExternalInput")
    out_d = nc.dram_tensor("out16", [B, JC, P, FSL], f16, kind="ExternalOutput")
    cv_d = nc.dram_tensor("cvec32", [FSL, B], f32, kind="ExternalOutput")

    with tile.TileContext(nc) as tc:
        with (
            tc.tile_pool(name="sing", bufs=1) as sing,
            tc.tile_pool(name="pps", bufs=1, space="PSUM") as pps,
            tc.tile_pool(name="pp", bufs=2, space="PSUM") as pp,
            tc.tile_pool(name="pss", bufs=2, space="PSUM") as pss,
            tc.tile_pool(name="pws", bufs=1, space="PSUM") as pws,
            tc.tile_pool(name="ppo", bufs=2, space="PSUM") as ppo,
            tc.tile_pool(name="dram", bufs=1, space="DRAM") as dram,
        ):
            # ---- persistent SBUF tiles ----
            x_sb = sing.tile([P, DC, B, S], f16)          # 32KB/part
            red = sing.tile([P, DC, B, 1024], f16)        # xs tree scratch
            wv_sb = sing.tile([P, G, DC, D], f16)         # lhsT [e, d] per (g,ec)
            wq_sb = sing.tile([P, DC, H, D], f16)         # lhsT [a, e] per (h,ac)
            wk_sb = sing.tile([P, DC, D], f16)            # lhsT [d, k] per (kc,dc)
            wo_sb = sing.tile([P, DC, G, FSL], f16)       # rhs [e, f] per (g,ec)
            bq_sb = sing.tile([P, DC, H], f16)            # rhs [k, h]
            bk_sb = sing.tile([P, DC], f32)
            bkS_sb = sing.tile([P, DC], f32)
            bv16 = sing.tile([P, DC, G], f16)
            bvo_sb = sing.tile([FSL, G], f32)
            bo_sb = sing.tile([FSL, 1], f32)
            cvec_sb = sing.tile([FSL, B], f32)
            ones_sb = sing.tile([P, P], f16)
            xs32 = sing.tile([P, DC, B], f32)
            xs16 = sing.tile([P, DC, B], f16)
            ksum16 = sing.tile([P, DC, B], f16)
            wqe_loc = sing.tile([P, DC, B, H], f16)
            bqd_loc = sing.tile([B, H], f16)
            wqe_all = sing.tile([P, G, DC, B, H], f16)
            bqd_all = sing.tile([1, B, G, H], f16)
            s1_sb = sing.tile([P, B, JC, G, H], f32)      # exp(scores)
            den_sb = sing.tile([P, B, JC, H], f32)
            rec_sb = sing.tile([P, B, JC, H], f32)
            w16_sb = sing.tile([P, B, JC, G, H], f16)     # softmax weights
            wsum_sb = sing.tile([1, B, G], f32)
            ws16_sb = sing.tile([1, B * G], f16)
            wsum_bc = sing.tile([P, B, G], f32)
            p16 = sing.tile([P, G, DC, FSL], f16)         # P_g[:, fslice]
            m16 = sing.tile([P, B, DC, FSL], f16)         # M[b][:, fslice]
            out_sb = sing.tile([P, JC, B, FSL], f16)

            # ---- internal DRAM (collective bounce) ----
            wq_bounce = dram.tile([CHUNK], f16)
            wq_gath = dram.tile([G * CHUNK], f16)

            nc.vector.memset(ones_sb[:, :], 1.0)

            # ---- input DMAs, ordered for the critical path:
            #      x chunks (xs tree), wk, wq  ->  AllGather chain
            #      wv, wo, biases              ->  P / cvec path
            nc.sync.dma_start(
                out=wq_sb[:, :, :, :], in_=wq_d.rearrange("(ac p) h e -> p ac h e", p=P)
            )
            for dc in range(DC):
                for hh in range(2):
                    nc.sync.dma_start(
                        out=x_sb[:, dc, :, hh * 1024:(hh + 1) * 1024],
                        in_=xT_d[dc * P:(dc + 1) * P, :, hh * 1024:(hh + 1) * 1024],
                    )
            nc.sync.dma_start(
                out=wk_sb[:, :, :], in_=wk_d.rearrange("(dc p) k -> p dc k", p=P)
            )
            nc.sync.dma_start(
                out=bk_sb[:, :], in_=bk_d.rearrange("(dc p) -> p dc", p=P)
            )
            nc.sync.dma_start(
                out=bq_sb[:, :, :], in_=bq_d.rearrange("(kc p) h -> p kc h", p=P)
            )

            # ---- A. xs[b,d] = sum_s x : fp16 halving tree per (dc, s-half) ----
            for dc in range(DC):
                for hh in range(2):
                    hb = hh * 1024
                    rb = hh * 512
                    nc.vector.tensor_tensor(
                        out=red[:, dc, :, rb:rb + 512],
                        in0=x_sb[:, dc, :, hb:hb + 512],
                        in1=x_sb[:, dc, :, hb + 512:hb + 1024],
                        op=mybir.AluOpType.add,
                    )
                    w = 256
                    while w >= 8:
                        nc.vector.tensor_tensor(
                            out=red[:, dc, :, rb:rb + w],
                            in0=red[:, dc, :, rb:rb + w],
                            in1=red[:, dc, :, rb + w:rb + 2 * w],
                            op=mybir.AluOpType.add,
                        )
                        w //= 2
            nc.vector.tensor_reduce(
                out=xs32[:, :, :],
                in_=red[:, :, :, :].rearrange(
                    "p dc b (hh o) -> p dc b hh o", hh=2
                )[:, :, :, :, 0:8],
                axis=mybir.AxisListType.XY,
                op=mybir.AluOpType.add,
            )
            nc.vector.tensor_copy(xs16[:, :, :], xs32[:, :, :])

            # ---- B. ksumT[k,b] = Wk_c^T xs + S*bk ----
            nc.vector.tensor_scalar_mul(bkS_sb[:, :], bk_sb[:, :], float(S))
            psmall = pps.tile([P, 512], f32, tag="small")
            psum_k = psmall[:, 0:8].rearrange("p (kc b) -> p kc b", kc=DC)
            for kc in range(DC):
                for dc in range(DC):
                    nc.tensor.matmul(
                        psum_k[:, kc, :],
                        lhsT=wk_sb[:, dc, kc * P:(kc + 1) * P],
                        rhs=xs16[:, dc, :],
                        start=(dc == 0),
                        stop=(dc == DC - 1),
                    )
            bk_b = bkS_sb[:, :]
            nc.vector.tensor_tensor(
                out=ksum16[:, :, :],
                in0=psum_k[:, :, :],
                in1=bass.AP(
                    tensor=bk_b.tensor, offset=bk_b.offset,
                    ap=list(bk_b.ap) + [[0, B]],
                ),
                op=mybir.AluOpType.add,
            )

            # ---- C. wq_eff[e,(b)] per (h, ec); bqdot[b,h]; scale; bounce ----
            psum_wq = psmall[:, 8:40].rearrange(
                "p (ec b h) -> p ec b h", ec=DC, b=B
            )
            for h in range(H):
                for ec in range(DC):
                    for kc in range(DC):
                        nc.tensor.matmul(
                            psum_wq[:, ec, :, h],
                            lhsT=wq_sb[:, kc, h, ec * P:(ec + 1) * P],
                            rhs=ksum16[:, kc, :],
                            start=(kc == 0),
                            stop=(kc == DC - 1),
                        )
            psum_bqd = psmall[0:B, 40:44]
            for kc in range(DC):
                nc.tensor.matmul(
                    psum_bqd[:, :],
                    lhsT=ksum16[:, kc, :],
                    rhs=bq_sb[:, kc, :],
                    start=(kc == 0),
                    stop=(kc == DC - 1),
                )
            nc.vector.tensor_scalar_mul(wqe_loc[:, :, :, :], psum_wq[:, :, :, :], INV_SQRT_D)
            nc.vector.tensor_scalar_mul(bqd_loc[:, :], psum_bqd[:, :], INV_SQRT_D)
            nc.sync.dma_start(
                out=wq_bounce[0:D * B * H].rearrange(
                    "(p ac b h) -> p ac b h", p=P, ac=DC, b=B
                ),
                in_=wqe_loc[:, :, :, :],
            )
            nc.sync.dma_start(
                out=wq_bounce[D * B * H:CHUNK].rearrange("(b h) -> b h", b=B),
                in_=bqd_loc[:, :],
            )

            # ---- D2. weight DMAs for the P path (the AllGather bounce slots
            #      between the 1MB chunks) ----
            for gp in range(4):
                nc.sync.dma_start(
                    out=wv_sb[:, 2 * gp:2 * gp + 2, :, :],
                    in_=wvT_d[2 * gp:2 * gp + 2, :, :].rearrange(
                        "g (ec p) d -> p g ec d", p=P
                    ),
                )
            nc.sync.dma_start(
                out=wo_sb[:, :, :, :],
                in_=wo_d.rearrange("(ec p) g f -> p ec g f", p=P),
            )
            nc.gpsimd.dma_start(
                out=bv16[:, :, :], in_=bv_d.rearrange("(ec p) g -> p ec g", p=P)
            )
            nc.sync.dma_start(
                out=bo_sb[:, :], in_=bo_d.rearrange("(f o) -> f o", o=1)
            )

            # ---- D. AllGather of (wq_eff, bqdot), fp16 ----
            nc.gpsimd.collective_compute(
                "AllGather",
                mybir.AluOpType.bypass,
                replica_groups=[list(range(N_CORES))],
                ins=[wq_bounce[:].opt()],
                outs=[wq_gath[:].opt()],
            )

            # ---- E. spread gathered results ----
            gap = wq_gath[:]
            nc.sync.dma_start(
                out=wqe_all[:, :, :, :, :],
                in_=bass.AP(
                    tensor=gap.tensor,
                    offset=gap.offset,
                    ap=[[DC * B * H, P], [CHUNK, G], [1, DC * B * H]],
                ),
            )
            nc.sync.dma_start(
                out=bqd_all[:, :, :, :],
                in_=bass.AP(
                    tensor=gap.tensor,
                    offset=gap.offset + D * B * H,
                    ap=[[0, 1], [H, B], [CHUNK, G], [1, H]],
                ),
            )

            # ---- F. P_g = Wv_g @ Wo_g[:, fsl]  (all groups, f-slice) ----
            for g in range(G):
                psum_p = pp.tile([P, DC, FSL], f32, tag="pp")
                for dc in range(DC):
                    for ec in range(DC):
                        nc.tensor.matmul(
                            psum_p[:, dc, :],
                            lhsT=wv_sb[:, g, ec, dc * P:(dc + 1) * P],
                            rhs=wo_sb[:, ec, g, :],
                            start=(ec == 0),
                            stop=(ec == DC - 1),
                        )
                nc.scalar.activation(
                    out=p16[:, g, :, :],
                    in_=psum_p[:, :, :],
                    func=mybir.ActivationFunctionType.Copy,
                )

            # ---- F2. bvo[f, g] = bv_g @ Wo_g[:, fsl]  (early) ----
            psum_bvo = psmall[0:FSL, 192:200]
            for g in range(G):
                for ec in range(DC):
                    nc.tensor.matmul(
                        psum_bvo[:, g:g + 1],
                        lhsT=wo_sb[:, ec, g, :],
                        rhs=bv16[:, ec, g:g + 1],
                        start=(ec == 0),
                        stop=(ec == DC - 1),
                    )
            nc.vector.tensor_copy(bvo_sb[:, :], psum_bvo[:, :])

            # ---- G. scores + exp + softmax + wsum (full sequence) ----
            for b in range(B):
                psum_s = pss.tile([P, JC, G * H], f32, tag="ps")
                bq_b = bqd_all[:, b, :, :]
                nc.tensor.matmul(
                    psum_s[:, :, :],
                    lhsT=ones_sb[0:1, :],
                    rhs=bass.AP(
                        tensor=bq_b.tensor, offset=bq_b.offset,
                        ap=[list(bq_b.ap[0]), [0, JC]] + list(bq_b.ap[1:]),
                    ),
                    start=True,
                    stop=False,
                )
                for j in range(JC):
                    for dc in range(DC):
                        nc.tensor.matmul(
                            psum_s[:, j, :],
                            lhsT=x_sb[:, dc, b, j * P:(j + 1) * P],
                            rhs=wqe_all[:, :, dc, b, :],
                            start=False,
                            stop=(j == JC - 1 and dc == DC - 1),
                            skip_group_check=True,
                        )
                nc.scalar.activation(
                    out=s1_sb[:, b, :, :, :].rearrange("p j g h -> p j (g h)"),
                    in_=psum_s[:, :, :],
                    func=mybir.ActivationFunctionType.Exp,
                )
                nc.vector.tensor_reduce(
                    out=den_sb[:, b, :, :],
                    in_=s1_sb[:, b, :, :, :].rearrange("p j g h -> p j h g"),
                    axis=mybir.AxisListType.X,
                    op=mybir.AluOpType.add,
                )
                nc.vector.reciprocal(rec_sb[:, b, :, :], den_sb[:, b, :, :])
                rb = rec_sb[:, b, :, :]
                nc.vector.tensor_tensor(
                    out=w16_sb[:, b, :, :, :].rearrange("p j g h -> p j h g"),
                    in0=s1_sb[:, b, :, :, :].rearrange("p j g h -> p j h g"),
                    in1=bass.AP(
                        tensor=rb.tensor,
                        offset=rb.offset,
                        ap=list(rb.ap) + [[0, G]],
                    ),
                    op=mybir.AluOpType.mult,
                )
                psum_ws = pws.tile([1, JC * G * H], f32, tag="ws")
                nc.tensor.matmul(
                    psum_ws[:, :],
                    lhsT=ones_sb[:, 0:1],
                    rhs=w16_sb[:, b, :, :, :],
                    start=True,
                    stop=True,
                )
                nc.vector.tensor_reduce(
                    out=wsum_sb[:, b, :],
                    in_=psum_ws[:, :].rearrange("p (j g h) -> p g j h", j=JC, g=G),
                    axis=mybir.AxisListType.XY,
                    op=mybir.AluOpType.add,
                )

            # ---- H. broadcast wsum to all partitions via PE (per b) ----
            psum_wb = psmall[:, 44:60].rearrange("p (b g) -> p b g", b=B)
            for b in range(B):
                nc.vector.tensor_copy(ws16_sb[:, b * G:(b + 1) * G], wsum_sb[:, b, :])
                nc.tensor.matmul(
                    psum_wb[:, b, :], lhsT=ones_sb[0:1, :],
                    rhs=ws16_sb[:, b * G:(b + 1) * G],
                    start=True, stop=True,
                )
                nc.vector.tensor_copy(wsum_bc[:, b, :], psum_wb[:, b, :])

            # ---- H2. PE warm-up fillers: keep the tensor engine busy through
            #      the softmax/combine window so the out matmuls run at full
            #      clock (cheap redundant column-sums into a recycled bank) ----
            for _ in range(32):
                psum_fill = pss.tile([P, JC, G * H], f32, tag="ps")
                nc.tensor.matmul(
                    psum_fill[:, :, :],
                    lhsT=ones_sb[:, :],
                    rhs=w16_sb[:, 0, :, :, :],
                    start=True,
                    stop=True,
                )

            # ---- I. M[b] = sum_g wsum[b,g] * P_g ----
            mh = sing.tile([P, B, DC, FSL], f16)
            for b in range(B):
                nc.vector.tensor_scalar(
                    out=m16[:, b, :, :],
                    in0=p16[:, 0, :, :],
                    scalar1=wsum_bc[:, b, 0:1],
                    scalar2=None,
                    op0=mybir.AluOpType.mult,
                )
                nc.vector.tensor_scalar(
                    out=mh[:, b, :, :],
                    in0=p16[:, 4, :, :],
                    scalar1=wsum_bc[:, b, 4:5],
                    scalar2=None,
                    op0=mybir.AluOpType.mult,
                )
                for g in (1, 2, 3):
                    nc.vector.scalar_tensor_tensor(
                        out=m16[:, b, :, :],
                        in0=p16[:, g, :, :],
                        scalar=wsum_bc[:, b, g:g + 1],
                        in1=m16[:, b, :, :],
                        op0=mybir.AluOpType.mult,
                        op1=mybir.AluOpType.add,
                    )
                    nc.vector.scalar_tensor_tensor(
                        out=mh[:, b, :, :],
                        in0=p16[:, g + 4, :, :],
                        scalar=wsum_bc[:, b, g + 4:g + 5],
                        in1=mh[:, b, :, :],
                        op0=mybir.AluOpType.mult,
                        op1=mybir.AluOpType.add,
                    )
                nc.vector.tensor_tensor(
                    out=m16[:, b, :, :],
                    in0=m16[:, b, :, :],
                    in1=mh[:, b, :, :],
                    op=mybir.AluOpType.add,
                )

            # ---- J. cvec[b] = sum_g wsum[b,g]*bvo[:,g] + bo; flip to [1,(b f)] ----
            for b in range(B):
                nc.vector.scalar_tensor_tensor(
                    out=cvec_sb[:, b:b + 1],
                    in0=bvo_sb[:, 0:1],
                    scalar=wsum_bc[0:FSL, b, 0:1],
                    in1=bo_sb[:, :],
                    op0=mybir.AluOpType.mult,
                    op1=mybir.AluOpType.add,
                )
                for g in range(1, G):
                    nc.vector.scalar_tensor_tensor(
                        out=cvec_sb[:, b:b + 1],
                        in0=bvo_sb[:, g:g + 1],
                        scalar=wsum_bc[0:FSL, b, g:g + 1],
                        in1=cvec_sb[:, b:b + 1],
                        op0=mybir.AluOpType.mult,
                        op1=mybir.AluOpType.add,
                    )
            nc.sync.dma_start(out=cv_d[:, :], in_=cvec_sb[:, :])

            # ---- K. out[b, s, fsl] = x[b] @ M[b] + cvec  (s on partitions) ----
            for b in range(B):
                for hf in range(2):
                    psum_o = ppo.tile([P, 8, FSL], f32, tag="po")
                    for jj in range(8):
                        j = hf * 8 + jj
                        for dc in range(DC):
                            nc.tensor.matmul(
                                psum_o[:, jj, :],
                                lhsT=x_sb[:, dc, b, j * P:(j + 1) * P],
                                rhs=m16[:, b, dc, :],
                                start=(dc == 0),
                                stop=(dc == DC - 1),
                            )
                    if hf == 0:
                        nc.scalar.activation(
                            out=out_sb[:, hf * 8:(hf + 1) * 8, b, :],
                            in_=psum_o[:, :, :],
                            func=mybir.ActivationFunctionType.Identity,
                        )
                    else:
                        nc.vector.tensor_copy(
                            out_sb[:, hf * 8:(hf + 1) * 8, b, :],
                            psum_o[:, :, :],
                        )
                    nc.sync.dma_start(
                        out=out_d[b, hf * 8:(hf + 1) * 8, :, :].rearrange(
                            "j p f -> p j f"
                        ),
                        in_=out_sb[:, hf * 8:(hf + 1) * 8, b, :],
                    )

    nc.compile()
    return nc


def kernel(x, Wq, bq, Wk, bk, Wv, bv, Wo, bo):
    from concourse.bass_utils import run_bass_kernel_spmd

    if "nc" not in _cache:
        _cache["nc"] = _build_nc()
    nc = _cache["nc"]

    f16 = np.float16
    xT16 = np.ascontiguousarray(
        np.asarray(x, np.float32).transpose(2, 0, 1)).astype(f16)  # [d,b,s]
    wq_r = np.asarray(Wq, np.float32).reshape(D, G, H, D)
    wvT16 = np.ascontiguousarray(
        np.asarray(Wv, np.float32).reshape(D, G, D).transpose(1, 2, 0)
    ).astype(f16)                                                   # [g,e,d]
    wo_r = np.asarray(Wo, np.float32).reshape(G, D, D)
    bq_r = np.asarray(bq, np.float32).reshape(G, H, D)
    in_maps = []
    for c in range(N_CORES):
        fs = slice(c * FSL, (c + 1) * FSL)
        in_maps.append({
            "xT16": xT16,
            "wk16": np.ascontiguousarray(
                np.asarray(Wk, np.float32)[:, c * D:(c + 1) * D]).astype(f16),
            "wq16": np.ascontiguousarray(
                wq_r[:, c].transpose(2, 1, 0)).astype(f16),          # [a,h,e]
            "wvT16": wvT16,
            "wo16": np.ascontiguousarray(
                wo_r[:, :, fs].transpose(1, 0, 2)).astype(f16),      # [e,g,f]
            "bq16": np.ascontiguousarray(bq_r[c].T).astype(f16),     # [k,h]
            "bk32": np.ascontiguousarray(
                np.asarray(bk, np.float32)[c * D:(c + 1) * D]),
            "bv32": np.ascontiguousarray(
                np.asarray(bv, np.float32).reshape(G, D).T),         # [e,g]
            "bo32": np.ascontiguousarray(np.asarray(bo, np.float32)[fs]),
        })
    res = run_bass_kernel_spmd(nc, in_maps, core_ids=list(range(N_CORES)))
    _cache["last_results"] = res
    full = np.concatenate(
        [r["out16"].reshape(B, S, FSL) for r in res.results], axis=2
    ).astype(np.float32)                              # [B, S, D]
    cvec = np.concatenate(
        [r["cvec32"].T for r in res.results], axis=1
    )                                                 # [B, D]
    return full + cvec[:, None, :]
